# revision 1
# baseline (speedup 1.0000x reference)
"""Trainium2 Bass kernel for nn_CompressionLayer (grouped per-chunk Linear + ReLU).

Math: x [256,512,512] is split into 16x16 chunks (N=1024 of them, a 32x32 grid);
each chunk n has its own Linear W[n] [64,256] + b[n]; y = relu(xc @ W^T + b),
recombined to [256, 65536].

Sharding: chunk-row parallelism over 8 NeuronCores — core c owns H rows
[64c, 64c+64) = chunk-rows 4c..4c+3 (128 chunks), the full batch, and writes
columns [8192c, 8192(c+1)) of the flat output. Per-core traffic: 32MB x + 8MB W
+ 8MB out.

Device kernel (per core, fp32-exact):
  - Host pre-packs x and W into kin-major (transposed) layouts so the PE needs
    no on-chip transposes; contraction (kin=256 = 2x128) sits on partitions.
  - Per chunk: psum[128b, 64o] = sum_h Xt[h][128k,128b].T-stationary @ Wt[h][128k,64o]
    with the bias added by a K=1 ones-matmul that initializes each psum bank
    (8 chunks share one [128,512] bank).
  - ScalarE applies ReLU while scattering psum into an output-ordered SBUF tile;
    one 1MB DMA per (b-tile, chunk-row) stores it.
All fp32 (hardware rel err ~1e-7 vs fp64 reference).
"""
from contextlib import ExitStack

import numpy as np

import concourse.bass as bass
import concourse.tile as tile
from concourse import bacc, mybir
from concourse._compat import with_exitstack
from concourse.bass_utils import run_bass_kernel_spmd

F32 = mybir.dt.float32

B, H, W = 256, 512, 512
N_CORES = 8
N_ILOC = 4       # chunk-rows per core
N_J = 32         # chunks per chunk-row
KOUT = 64
JG = 8           # chunks per psum-bank group
N_G = N_J // JG


@with_exitstack
def _build(ctx: ExitStack, tc, out, xt, wt, bkp):
    nc = tc.nc
    xt_pool = ctx.enter_context(tc.tile_pool(name="xt", bufs=3))
    wt_pool = ctx.enter_context(tc.tile_pool(name="wt", bufs=2))
    asm_pool = ctx.enter_context(tc.tile_pool(name="asm", bufs=4))
    const_pool = ctx.enter_context(tc.tile_pool(name="const", bufs=1))
    py_pool = ctx.enter_context(tc.tile_pool(name="py", bufs=8, space="PSUM"))

    ones = const_pool.tile([1, 128], F32)
    nc.vector.memset(ones, 1.0)
    bk_sb = const_pool.tile([1, 8192], F32)
    nc.sync.dma_start(bk_sb[:], bkp[:])

    for il in range(N_ILOC):
        wt_t = wt_pool.tile([128, 4096], F32, tag="wt")
        nc.scalar.dma_start(wt_t[:], wt[il])
        xh = []
        for h in range(2):
            t = xt_pool.tile([128, 8192], F32, tag="xt", name=f"xt{h}")
            nc.sync.dma_start(t[:], xt[il, h])
            xh.append(t)

        for bt in range(2):
            asm = asm_pool.tile([128, 2048], F32, tag="asm")
            for g in range(N_G):
                py = py_pool.tile([128, 512], F32, tag="py")
                # bias init: psum[b, j_loc*64+o] = bias[(8g+j_loc)*64+o]
                nc.tensor.matmul(
                    py[:], ones[:],
                    bk_sb[:, (il * N_J + g * JG) * KOUT:(il * N_J + (g + 1) * JG) * KOUT],
                    start=True, stop=False,
                )
                for j_loc in range(JG):
                    j = g * JG + j_loc
                    for h in range(2):
                        nc.tensor.matmul(
                            py[:, j_loc * KOUT:(j_loc + 1) * KOUT],
                            xh[h][:, j * 256 + bt * 128: j * 256 + bt * 128 + 128],
                            wt_t[:, j * 128 + h * 64: j * 128 + h * 64 + KOUT],
                            start=False, stop=(j_loc == JG - 1 and h == 1),
                        )
                # ReLU + scatter into output-ordered tile: asm[b, oh*256 + j*8 + ow]
                out_ap = bass.AP(
                    tensor=asm.tensor,
                    offset=asm.offset + g * 64,
                    ap=[asm.ap[0], [8, JG], [256, 8], [1, 8]],
                )
                nc.scalar.activation(
                    out_ap,
                    py[:].rearrange("p (j oh ow) -> p j oh ow", j=JG, oh=8),
                    mybir.ActivationFunctionType.Relu,
                )
            nc.gpsimd.dma_start(
                out[bt * 128:(bt + 1) * 128, il * 2048:(il + 1) * 2048], asm[:]
            )


_NC_CACHE = None


def _get_nc():
    global _NC_CACHE
    if _NC_CACHE is None:
        nc = bacc.Bacc("TRN2", target_bir_lowering=False, debug=False)
        xt = nc.dram_tensor("xt", [4, 2, 128, 8192], F32, kind="ExternalInput").ap()
        wt = nc.dram_tensor("wt", [4, 128, 4096], F32, kind="ExternalInput").ap()
        bkp = nc.dram_tensor("bkp", [1, 8192], F32, kind="ExternalInput").ap()
        out = nc.dram_tensor("out", [256, 8192], F32, kind="ExternalOutput").ap()
        with tile.TileContext(nc) as tc:
            _build(tc, out, xt, wt, bkp)
        nc.compile()
        _NC_CACHE = nc
    return _NC_CACHE


def _repack_core(x, Wk, bk, c):
    xs = x[:, 64 * c:64 * (c + 1), :]                     # [256, 64, 512]
    # xt[il][h][p=(kh2*16+kw)][j*256+b] = xs[b, il*16 + h*8 + kh2, j*16 + kw]
    xtp = xs.reshape(B, 4, 2, 8, 32, 16).transpose(1, 2, 3, 5, 4, 0)
    xtp = np.ascontiguousarray(xtp).reshape(4, 2, 128, 32 * B)

    ws = Wk[128 * c:128 * (c + 1)]                        # [128, 64, 256]
    # wt[il][p=k%128][j*128 + h*64 + o] = ws[il*32+j, o, h*128+p]
    wtp = ws.reshape(4, 32, 64, 2, 128).transpose(0, 4, 1, 3, 2)
    wtp = np.ascontiguousarray(wtp).reshape(4, 128, 4096)

    bkp = np.ascontiguousarray(bk[128 * c:128 * (c + 1)].reshape(1, 8192))
    return {"xt": xtp, "wt": wtp, "bkp": bkp}


def kernel(x, Wk, bk):
    x = np.ascontiguousarray(np.asarray(x, dtype=np.float32))
    Wk = np.ascontiguousarray(np.asarray(Wk, dtype=np.float32))
    bk = np.ascontiguousarray(np.asarray(bk, dtype=np.float32))
    assert x.shape == (B, H, W) and Wk.shape == (1024, 64, 256) and bk.shape == (1024, 64)

    in_maps = [_repack_core(x, Wk, bk, c) for c in range(N_CORES)]
    nc = _get_nc()
    res = run_bass_kernel_spmd(nc, in_maps, core_ids=list(range(N_CORES)))
    return np.concatenate([res.results[c]["out"] for c in range(N_CORES)], axis=1)
